# revision 43
# baseline (speedup 1.0000x reference)
"""Multi-head attention (B=2, S=2048, H=1024, 16 heads) on 8 TRN2 NeuronCores.

Sharding: data-parallel over batch (2) x tensor-parallel over heads (16 -> 4
groups of 4 heads).  Core c = b*4 + g handles batch b, heads [4g, 4g+4).

Per-core math (fp16 storage / fp32 accumulate), with x = q|k|v of its batch:
  QP_T[d, s] = (Wq_g x^T + bq_g)   stored transposed, d on partitions
  KP_T[d, s] = (Wk_g x^T + bk_g)
  VP[s, d]   = x Wv_g^T            natural layout  (bv folded on host)
  per head-pair p (heads 2p, 2p+1), i-block ib (512 queries):
    S_T[j, i] = KP_T_h^T-contracted scores (d contracts), row-tiled pair
    A = exp(S_T / 8)   softmax w/o max-sub; most tiles on the scalar engine
                       (exact), a tunable subset on the vector engine via a
                       Schraudolph-style int16 bitcast exp (~2% rms)
    AV via 4x 128x32 column-tiled matmuls per j-tile: head A dims -> PSUM
       partitions 0-63, head B dims -> 64-127, all four running concurrently
       (2x the full-array m=65 layout).
    L via a separate column-tiled rider pass: per j-tile, four concurrent
       m=1 matmuls over 256-query halves: (A,q-lo)@0 (B,q-lo)@32 (A,q-hi)@64
       (B,q-hi)@96, accumulated over all j.
    O_norm = O * (1/L)   1/L = exp(-ln L) on ACT straight from PSUM (97
       partitions in parallel); PE broadcast via two row-tiled selector
       matmuls into separate PSUM banks.
  out_T[o, i] = Wo_g^T-contracted projection of O_norm   -> [1024, 2048] f32
Host: out[b] = sum_g out_T(b,g)^T + (Wo @ bv + bo).
"""

import json
import math

import numpy as np

S = 2048
H = 1024
DL = 256          # local projection dim = 4 heads * 64
P = 128
HD = 64
NK = H // P       # 8 k-tiles over hidden dim
NI = 4            # i blocks of 512 queries
NJ = S // P       # 16 j tiles of 128 keys
NB = 512          # free-dim block

# Schraudolph fp16-bitcast exp:  i16 = round(s * A_SCH + B_SCH); bits are
# fp16(exp(s/8)).  A folds the 1/sqrt(hd)=0.125 score scale.
A_SCH = 0.125 * 1024.0 * 1.4426950408889634
B_SCH = 15.0 * 1024.0 - 60.0

_nc_cache = {}


# --------------------------------------------------------------------------
# BIR fix: this container's walrus supports only ONE sync wait (and update)
# per TPB instruction; Tile attaches several.  Split extras onto single-wait
# EventSemaphore instructions at the serialization boundary.
# --------------------------------------------------------------------------
_wsplit_counter = [0]


def _mk_evsem(engine, debug, wait=None, update=None):
    _wsplit_counter[0] += 1
    return {
        "debug": debug,
        "engine": engine,
        "ins": [],
        "outs": [],
        "name": f"wsplit-{_wsplit_counter[0]}",
        "opcode": "EventSemaphore",
        "sync_info": {
            "on_wait": [wait] if wait else [],
            "on_update": [update] if update else [],
        },
    }


def _split_bir_waits(bir):
    for f in bir.get("functions", []):
        for blk in f.get("blocks", []):
            out = []
            for inst in blk.get("instructions", []):
                si = inst.get("sync_info")
                waits = list(si.get("on_wait") or []) if si else []
                updates = list(si.get("on_update") or []) if si else []
                eng = inst.get("engine")
                dbg = inst.get("debug", 0)
                if len(waits) > 1:
                    for w in waits[:-1]:
                        out.append(_mk_evsem(eng, dbg, wait=w))
                    si["on_wait"] = [waits[-1]]
                out.append(inst)
                if len(updates) > 1:
                    si["on_update"] = [updates[0]]
                    for u in updates[1:]:
                        out.append(_mk_evsem(eng, dbg, update=u))
            blk["instructions"] = out
    return bir


def _install_bir_fix():
    import concourse.bass as bass

    if getattr(bass.Bass, "_wsplit_installed", False):
        return
    orig = bass.Bass.to_json_bytes

    def to_json_bytes(self, *a, **k):
        bir = json.loads(orig(self, *a, **k))
        return json.dumps(_split_bir_waits(bir)).encode()

    bass.Bass.to_json_bytes = to_json_bytes
    bass.Bass._wsplit_installed = True


# --------------------------------------------------------------------------
# Kernel builder
# --------------------------------------------------------------------------

def _build_nc():
    import concourse.bass as bass
    import concourse.mybir as mybir
    import concourse.tile as tile

    f16 = mybir.dt.float16
    i16 = mybir.dt.int16
    f32 = mybir.dt.float32
    f32r = mybir.dt.float32r
    Exp = mybir.ActivationFunctionType.Exp
    Ln = mybir.ActivationFunctionType.Ln

    nc = bass.Bass("TRN2")

    xq = nc.dram_tensor("xq", [H, S], f16, kind="ExternalInput")
    xk = nc.dram_tensor("xk", [H, S], f16, kind="ExternalInput")
    xv = nc.dram_tensor("xv", [H, S], f16, kind="ExternalInput")
    wqT = nc.dram_tensor("wqT", [H, DL], f16, kind="ExternalInput")
    wkT = nc.dram_tensor("wkT", [H, DL], f16, kind="ExternalInput")
    wvT = nc.dram_tensor("wvT", [H, DL], f16, kind="ExternalInput")
    woT = nc.dram_tensor("woT", [DL, H], f16, kind="ExternalInput")
    bias = nc.dram_tensor("bias", [P, 4], f32, kind="ExternalInput")  # bq0 bq1 bk0 bk1
    out = nc.dram_tensor("out", [H, S], f16, kind="ExternalOutput")

    with tile.TileContext(nc) as tc:
        with (
            tc.tile_pool(name="persist", bufs=1) as persist,
            tc.tile_pool(name="xpool", bufs=48) as xpool,
            tc.tile_pool(name="exppool", bufs=10) as exppool,
            tc.tile_pool(name="lrpool", bufs=3) as lrpool,
            tc.tile_pool(name="oevpool", bufs=2) as oevpool,
            tc.tile_pool(name="evpool", bufs=3) as evpool,
            tc.tile_pool(name="scps", bufs=2, space="PSUM") as scps,
            tc.tile_pool(name="avps", bufs=1, space="PSUM") as avps,
            tc.tile_pool(name="lps", bufs=1, space="PSUM") as lpsp,
            tc.tile_pool(name="auxps", bufs=2, space="PSUM") as auxps,
        ):
            # ---- persistent tiles ----
            wq_sb = persist.tile([P, NK, DL], f16, name="wq_sb")
            wk_sb = persist.tile([P, NK, DL], f16, name="wk_sb")
            wv_sb = persist.tile([P, NK, DL], f16, name="wv_sb")
            wo_sb = persist.tile([P, 2, H], f16, name="wo_sb")
            bias_sb = persist.tile([P, 4], f32, name="bias_sb")
            sel4 = persist.tile([P, P], f32r, name="sel4")
            ones16 = persist.tile([P, NB], f16, name="ones16")
            qpt = persist.tile([P, 2, S], f16, name="qpt")
            kpt = persist.tile([P, 2, S], f16, name="kpt")
            vp = persist.tile([P, NJ, DL], f16, name="vp")
            onorm = persist.tile([P, 2, S], f16, name="onorm")

            nc.sync.dma_start(wv_sb[:], wvT.rearrange("(k p) d -> p k d", p=P))
            nc.sync.dma_start(wk_sb[:], wkT.rearrange("(k p) d -> p k d", p=P))
            nc.sync.dma_start(wq_sb[:], wqT.rearrange("(k p) d -> p k d", p=P))
            nc.sync.dma_start(bias_sb[:], bias[:])
            # selector for the 1/L broadcast: rows 0/64 (query halves) ->
            # out partitions 0-63 (head A), rows 32/96 -> 64-127 (head B)
            sel_f = evpool.tile([P, P], f32, name="self_t")
            nc.gpsimd.memset(sel_f[:], 0.0)
            nc.gpsimd.memset(sel_f[0:1, 0:HD], 1.0)
            nc.gpsimd.memset(sel_f[32:33, HD:P], 1.0)
            nc.gpsimd.memset(sel_f[64:65, 0:HD], 1.0)
            nc.gpsimd.memset(sel_f[96:97, HD:P], 1.0)
            nc.vector.tensor_copy(sel4[:], sel_f[:])
            nc.gpsimd.memset(ones16[:], 1.0)

            # ---- PE warm-up: dense junk matmuls during the DMA lead-in so
            # the HAM clock-gate releases before real work arrives ----
            # long enough to span the DMA lead-in (~17us to first x tile) so
            # HAM is warm when the projections start: ~32 cold matmuls trip
            # the un-throttle at ~3.4us, the rest run warm at ~56ns
            warm_ps = auxps.tile([P, NB], f32, name="aux_t")
            for w in range(120):
                nc.tensor.matmul(
                    warm_ps[:, 0:P], ones16[:, 0:P], ones16[:, 0:P],
                    start=(w == 0), stop=(w == 119),
                )

            # one-time fill of the L bank so rows never written by the L
            # matmuls hold a safe value (recip stays finite, selector is 0
            # there anyway)
            lp_init = lpsp.tile([P, NB], f32, name="lp_t")
            nc.vector.memset(lp_init[:], 1.0)

            # ---- x loads: [P, 2*NB] halves keyed (k, h) — 256 KB per DMA
            # descriptor (the Sync queue issues descriptors serially, so
            # fewer/larger transfers raise the input stream rate), all of x
            # resident.  (k, n) quarter lookups return (tile, col offset).
            xv_q = {}
            xk_q = {}
            xq_q = {}
            def load_h(dst, src, k, h, eng):
                t = xpool.tile([P, 2 * NB], f16, name="x_sb")
                eng.dma_start(
                    t[:], src[k * P:(k + 1) * P, h * 2 * NB:(h + 1) * 2 * NB])
                dst[(k, 2 * h)] = (t, 0)
                dst[(k, 2 * h + 1)] = (t, NB)

            # all x loads on the sync hardware-dynamic DMA queue; splitting
            # across the gpsimd (software-dynamic, slow) or scalar queues
            # measured strictly worse
            load_order = [
                (xk_q, xk, 0, nc.sync), (xq_q, xq, 0, nc.sync),
                (xv_q, xv, 0, nc.sync), (xk_q, xk, 1, nc.sync),
                (xv_q, xv, 1, nc.sync), (xq_q, xq, 1, nc.sync),
            ]
            for dst, src, h, eng in load_order:
                for k in range(NK):
                    load_h(dst, src, k, h, eng)
            nc.sync.dma_start(wo_sb[:], woT.rearrange("(k p) d -> p k d", p=P))

            # ---- V projection for one s-tile ----
            def v_block(s):
                n, c = divmod(s, 4)
                ps = auxps.tile([P, NB], f32, name="aux_t")
                for k in range(NK):
                    t, off = xv_q[(k, n)]
                    nc.tensor.matmul(
                        ps[:, :DL],
                        t[:, off + c * P:off + (c + 1) * P],
                        wv_sb[:, k, :],
                        start=(k == 0),
                        stop=(k == NK - 1),
                    )
                nc.vector.tensor_copy(vp[:, s, :], ps[:, :DL])

            # ---- K/Q projection, one n-block ----
            def proj_qk_n(wsb, xq_, dst, bcol, m, n):
                ps = auxps.tile([P, NB], f32, name="aux_t")
                for k in range(NK):
                    t, off = xq_[(k, n)]
                    nc.tensor.matmul(
                        ps[:],
                        wsb[:, k, m * P:(m + 1) * P],
                        t[:, off:off + NB],
                        start=(k == 0),
                        stop=(k == NK - 1),
                    )
                nc.vector.tensor_scalar_add(
                    dst[:, m, n * NB:(n + 1) * NB], ps[:],
                    bias_sb[:, bcol + m:bcol + m + 1],
                )

            # ---- one Wo output column block (i-block n, out-rows mo*128) ----
            def wo_mo(n, mo, deep=False):
                # deep=True (final serial tail): rotate PSUM across scps too
                # (free by then) for a 4-deep MM->cast->DMA pipeline
                if deep and mo % 2:
                    psa = scps.tile([P, 2 * NB], f32, name="sc_t")[:, 0:NB]
                else:
                    psa = auxps.tile([P, NB], f32, name="aux_t")[:]
                for k2 in range(2):
                    nc.tensor.matmul(
                        psa,
                        wo_sb[:, k2, mo * P:(mo + 1) * P],
                        onorm[:, k2, n * NB:(n + 1) * NB],
                        start=(k2 == 0),
                        stop=(k2 == 1),
                    )
                ot = evpool.tile([P, NB], f16, name="ot_t")
                if mo % 2:
                    # alternate PSUM evacuation between the scalar and
                    # vector engines so neither becomes the gate
                    nc.scalar.copy(ot[:], psa)
                else:
                    nc.vector.tensor_copy(ot[:], psa)
                nc.sync.dma_start(out[mo * P:(mo + 1) * P, n * NB:(n + 1) * NB], ot[:])

            # ---- normalization tail of one finished (p, ib): returned as a
            # list of closures to be interleaved into the next ib ----
            def make_tail(p, ib, av, lp):
                isl = slice(ib * NB, (ib + 1) * NB)
                HB = NB // 2
                lnb = lrpool.tile([P, NB], f32, name="lnb_t")
                rbs = lrpool.tile([P, NB], f32r, name="rbs_t")
                o_ev = oevpool.tile([P, NB], f16, name="oev_t")
                st = {}

                def c_evac():
                    nc.vector.tensor_copy(o_ev[:], av[:])

                def c_ln():
                    # rows 0/32/64/96 hold L for (A,B) x (q-lo,q-hi); the
                    # 1.0-filled filler rows come out as ln=0 -> 1/L=1 and
                    # are zeroed by the selector
                    nc.scalar.activation(lnb[0:97, 0:HB], lp[0:97, 0:HB], Ln)

                def c_exp():
                    # 1/L = exp(-ln L) on the scalar engine (97 partitions
                    # in parallel; the DVE reciprocal instruction is ~4.5x
                    # slower per column)
                    with nc.allow_low_precision(
                            reason="softmax denom via f32r"):
                        nc.scalar.activation(
                            rbs[0:97, 0:HB], lnb[0:97, 0:HB], Exp, scale=-1.0)

                def c_bcast():
                    # two row-tiled selector matmuls (contract at partitions
                    # 0-32 and 64-96) -> separate PSUM banks, one per
                    # 256-query half
                    rb0 = auxps.tile([P, NB], f32, name="aux_t")
                    rb1 = auxps.tile([P, NB], f32, name="aux_t")
                    nc.tensor.matmul(
                        rb0[:, 0:HB], sel4[0:33, :], rbs[0:33, 0:HB],
                        start=True, stop=True,
                    )
                    nc.tensor.matmul(
                        rb1[:, 0:HB], sel4[64:97, :], rbs[64:97, 0:HB],
                        start=True, stop=True,
                    )
                    st["rb0"], st["rb1"] = rb0, rb1

                def c_mul():
                    i0 = ib * NB
                    nc.vector.tensor_mul(
                        onorm[:, p, i0:i0 + HB], o_ev[:, 0:HB],
                        st["rb0"][:, 0:HB])
                    nc.vector.tensor_mul(
                        onorm[:, p, i0 + HB:i0 + NB], o_ev[:, HB:NB],
                        st["rb1"][:, 0:HB])

                return [c_evac, c_ln, c_exp, c_bcast, c_mul]

            # ---- attention over one (head-pair p, i-block ib) ----
            def attention_ib(p, ib, v_inline=False, mids=None, dve_js=(),
                             carry=(), final=False):
                isl = slice(ib * NB, (ib + 1) * NB)
                av = avps.tile([P, NB], f32, name="av_t")
                lp = lpsp.tile([P, NB], f32, name="lp_t")
                carry = list(carry)
                es = []
                pend = []
                a0 = (2 * p) * HD
                b0 = (2 * p + 1) * HD

                def emit_av(jb):
                    eA, eB = es[jb][0], es[jb][1]
                    for ci, (voff, ee) in enumerate(
                            ((a0, eA), (a0 + 32, eA), (b0, eB), (b0 + 32, eB))):
                        nc.tensor.matmul(
                            av[ci * 32:(ci + 1) * 32, :],
                            vp[:, jb, voff:voff + 32],
                            ee,
                            start=(jb == 0), stop=(jb == NJ - 1),
                            tile_position=(0, ci * 32),
                        )

                def emit_L(jb):
                    # column-tiled m=1 rider pass over 256-query halves:
                    # (A,q-lo)@0 (B,q-lo)@32 (A,q-hi)@64 (B,q-hi)@96, each
                    # accumulating over all 16 j-tiles
                    HB = NB // 2
                    for ci, ee in enumerate(es[jb][2:6]):
                        nc.tensor.matmul(
                            lp[ci * 32:ci * 32 + 1, 0:HB],
                            ones16[:, 0:1],
                            ee,
                            start=(jb == 0), stop=(jb == NJ - 1),
                            tile_position=(0, ci * 32),
                        )

                for jj in range(0, NJ, 2):
                    for jb in (jj, jj + 1):
                        jsl = slice(jb * P, (jb + 1) * P)
                        sc = scps.tile([P, 2 * NB], f32, name="sc_t")
                        nc.tensor.matmul(
                            sc[:, 0:NB], kpt[0:HD, p, jsl], qpt[0:HD, p, isl],
                            start=True, stop=True,
                        )
                        nc.tensor.matmul(
                            sc[:, NB:2 * NB], kpt[HD:P, p, jsl], qpt[HD:P, p, isl],
                            start=True, stop=True,
                        )
                        HB = NB // 2
                        cuts = ((0, NB), (NB, 2 * NB), (0, HB), (NB, NB + HB),
                                (HB, NB), (NB + HB, 2 * NB))
                        if jb in dve_js:
                            e = exppool.tile([P, 2 * NB], i16, name="e_t")
                            nc.vector.tensor_scalar(
                                e[:], sc[:], float(A_SCH), float(B_SCH),
                                mybir.AluOpType.mult, mybir.AluOpType.add,
                            )
                            es.append(tuple(
                                e[:, lo:hi].bitcast(f16) for lo, hi in cuts))
                        else:
                            e = exppool.tile([P, 2 * NB], f16, name="e_t")
                            nc.scalar.activation(e[:], sc[:], Exp, scale=0.125)
                            es.append(tuple(
                                e[:, lo:hi] for lo, hi in cuts))
                        if v_inline and 4 <= jb:
                            v_block(jb - 2)
                        if mids is not None and jb in mids:
                            pend.extend(mids[jb])
                    # carried-in PE work (prev ib's trailing AV/L passes or
                    # lead-in v-blocks) fills the pipeline-refill bubble
                    for _ in range(4):
                        if carry:
                            carry.pop(0)()
                    # AV+L flush: one contiguous column-tiled run
                    if jj in (6, 10, 14):
                        base = jj - 6
                        for j2 in range(base, base + 4):
                            emit_av(j2)
                        for j2 in range(base, base + 4):
                            emit_L(j2)
                    if final and jj == 14:
                        emit_av(12)
                        emit_av(13)
                        emit_L(12)
                        emit_L(13)
                    # mid callbacks run at the block boundary, after the
                    # column-tiled flush, so proj/wo full-array matmuls do
                    # not fragment the row-tiled QK runs (each fragment
                    # costs a PE tiling-mode drain)
                    for cb in pend:
                        cb()
                    pend = []

                if v_inline:
                    v_block(NJ - 2)
                    v_block(NJ - 1)
                if final:
                    trailing = [lambda: emit_av(14), lambda: emit_av(15),
                                lambda: emit_L(14), lambda: emit_L(15)]
                else:
                    trailing = [
                        (lambda a=jb: emit_av(a))
                        for jb in range(NJ - 4, NJ)
                    ] + [(lambda a=jb: emit_L(a)) for jb in range(NJ - 4, NJ)]
                return make_tail(p, ib, av, lp), trailing

            # ---- schedule ----
            def P_(wsb, xd, dst, bcol, m, n):
                return lambda: proj_qk_n(wsb, xd, dst, bcol, m, n)

            K0 = lambda n: P_(wk_sb, xk_q, kpt, 2, 0, n)
            Q0 = lambda n: P_(wq_sb, xq_q, qpt, 0, 0, n)
            K1 = lambda n: P_(wk_sb, xk_q, kpt, 2, 1, n)
            Q1 = lambda n: P_(wq_sb, xq_q, qpt, 0, 1, n)

            def sched(tail, extra=None, wo_n=None):
                """mids dict: tail closures (evac@3, ln@4, exp@5, bcast@7,
                mul@9), wo pieces 2-per-slot at 12..15, plus extras.  The
                prev ib's av/lp only complete once its carried trailing
                passes ran (slots 0..3)."""
                m = {}
                if tail is not None:
                    slots = (3, 4, 5, 7, 9)
                    for i, cb in enumerate(tail):
                        m.setdefault(slots[i], []).append(cb)
                if wo_n is not None:
                    for mo in range(8):
                        m.setdefault(8 + mo, []).append(
                            (lambda n_, mo_: lambda: wo_mo(n_, mo_))(wo_n, mo))
                if extra:
                    for s, cbs in extra.items():
                        m.setdefault(s, []).extend(cbs)
                return m

            # DVE-exp tile positions chosen so the scalar engine never gets
            # more than two consecutive j-tiles (3 back-to-back ACT exps =
            # 3.3us serial vs ~1.7us of PE work -> per-ib PE stall)
            D5 = (2, 5, 8, 11, 13)    # final ib
            D5L = (2, 5, 8, 11, 14)   # non-wo ibs
            DW = (2, 5, 8, 11, 13, 15)  # wo ibs: one more DVE tile

            K0(0)()
            Q0(0)()
            t, cav = attention_ib(0, 0, v_inline=True, mids=sched(
                None, extra={2: [K0(1)], 5: [K0(2)], 8: [K0(3)], 11: [Q0(1)]}),
                carry=[lambda s=s: v_block(s) for s in range(2)])
            t, cav = attention_ib(0, 1, mids=sched(
                t, extra={5: [Q0(2)], 9: [K1(0)], 13: [K1(1)]}),
                dve_js=D5L, carry=cav)
            t, cav = attention_ib(0, 2, mids=sched(
                t, extra={7: [Q0(3)], 11: [Q1(0)], 14: [Q1(1)]}),
                dve_js=D5L, carry=cav)
            t, cav = attention_ib(1, 0, mids=sched(
                t, extra={2: [K1(2)], 5: [K1(3)], 9: [Q1(2)]}),
                dve_js=D5L, carry=cav)
            t, cav = attention_ib(1, 1, mids=sched(
                t, extra={9: [Q1(3)]}), dve_js=DW, carry=cav)
            t, cav = attention_ib(1, 2, mids=sched(t, wo_n=0), dve_js=DW,
                                  carry=cav)
            t, cav = attention_ib(1, 3, mids=sched(t, wo_n=1), dve_js=DW,
                                  carry=cav)
            t, cav = attention_ib(0, 3, mids=sched(t, wo_n=2), dve_js=D5,
                                  carry=cav, final=True)
            # final serial tail
            for cb in cav:
                cb()
            for cb in t:
                cb()
            for mo in range(8):
                wo_mo(3, mo, deep=True)

    return nc


def _get_nc():
    if "nc" not in _nc_cache:
        _install_bir_fix()
        _nc_cache["nc"] = _build_nc()
    return _nc_cache["nc"]


# --------------------------------------------------------------------------
# Host wrapper
# --------------------------------------------------------------------------
def run(inputs, trace=False):
    from concourse.bass_utils import run_bass_kernel_spmd

    q = np.asarray(inputs["q"], np.float32)
    k = np.asarray(inputs["k"], np.float32)
    v = np.asarray(inputs["v"], np.float32)
    Wq = np.asarray(inputs["Wq"], np.float32)
    bq = np.asarray(inputs["bq"], np.float32)
    Wk = np.asarray(inputs["Wk"], np.float32)
    bk = np.asarray(inputs["bk"], np.float32)
    Wv = np.asarray(inputs["Wv"], np.float32)
    bv = np.asarray(inputs["bv"], np.float32)
    Wo = np.asarray(inputs["Wo"], np.float32)
    bo = np.asarray(inputs["bo"], np.float32)

    nc = _get_nc()

    xT = {}
    for b in range(2):
        xT[b] = (
            np.ascontiguousarray(q[b].T).astype(np.float16),
            np.ascontiguousarray(k[b].T).astype(np.float16),
            np.ascontiguousarray(v[b].T).astype(np.float16),
        )

    in_maps = []
    for c in range(8):
        b, g = divmod(c, 4)
        sl = slice(g * DL, (g + 1) * DL)
        bias = np.stack(
            [bq[sl][:P], bq[sl][P:], bk[sl][:P], bk[sl][P:]], axis=1
        ).astype(np.float32)
        in_maps.append({
            "xq": xT[b][0],
            "xk": xT[b][1],
            "xv": xT[b][2],
            "wqT": np.ascontiguousarray(Wq[sl, :].T).astype(np.float16),
            "wkT": np.ascontiguousarray(Wk[sl, :].T).astype(np.float16),
            "wvT": np.ascontiguousarray(Wv[sl, :].T).astype(np.float16),
            "woT": np.ascontiguousarray(Wo[:, sl].T).astype(np.float16),
            "bias": bias,
        })

    res = run_bass_kernel_spmd(
        nc, in_maps, core_ids=list(range(8)), trace=trace,
    )
    outs = [r["out"] for r in res.results]

    const = (Wo @ bv + bo).astype(np.float32)  # [1024]
    full = np.empty((2, S, H), np.float32)
    for b in range(2):
        acc = outs[4 * b].astype(np.float32).copy()
        for g in range(1, 4):
            acc += outs[4 * b + g]
        full[b] = acc.T + const
    return full, res


def kernel(**inputs):
    full, _ = run(inputs, trace=False)
    return full


# revision 44
# speedup vs baseline: 1.0237x; 1.0237x over previous
"""Multi-head attention (B=2, S=2048, H=1024, 16 heads) on 8 TRN2 NeuronCores.

Sharding: data-parallel over batch (2) x tensor-parallel over heads (16 -> 4
groups of 4 heads).  Core c = b*4 + g handles batch b, heads [4g, 4g+4).

Per-core math (fp16 storage / fp32 accumulate), with x = q|k|v of its batch:
  QP_T[d, s] = (Wq_g x^T + bq_g)   stored transposed, d on partitions
  KP_T[d, s] = (Wk_g x^T + bk_g)
  VP[s, d]   = x Wv_g^T            natural layout  (bv folded on host)
  per head-pair p (heads 2p, 2p+1), i-block ib (512 queries):
    S_T[j, i] = KP_T_h^T-contracted scores (d contracts), row-tiled pair
    A = exp(S_T / 8)   softmax w/o max-sub; most tiles on the scalar engine
                       (exact), a tunable subset on the vector engine via a
                       Schraudolph-style int16 bitcast exp (~2% rms)
    AV via 4x 128x32 column-tiled matmuls per j-tile: head A dims -> PSUM
       partitions 0-63, head B dims -> 64-127, all four running concurrently
       (2x the full-array m=65 layout).
    L via a separate column-tiled rider pass: per j-tile, four concurrent
       m=1 matmuls over 256-query halves: (A,q-lo)@0 (B,q-lo)@32 (A,q-hi)@64
       (B,q-hi)@96, accumulated over all j.
    O_norm = O * (1/L)   1/L = exp(-ln L) on ACT straight from PSUM (97
       partitions in parallel); PE broadcast via two row-tiled selector
       matmuls into separate PSUM banks.
  out_T[o, i] = Wo_g^T-contracted projection of O_norm   -> [1024, 2048] f32
Host: out[b] = sum_g out_T(b,g)^T + (Wo @ bv + bo).
"""

import json
import math

import numpy as np

S = 2048
H = 1024
DL = 256          # local projection dim = 4 heads * 64
P = 128
HD = 64
NK = H // P       # 8 k-tiles over hidden dim
NI = 4            # i blocks of 512 queries
NJ = S // P       # 16 j tiles of 128 keys
NB = 512          # free-dim block

# Schraudolph fp16-bitcast exp:  i16 = round(s * A_SCH + B_SCH); bits are
# fp16(exp(s/8)).  A folds the 1/sqrt(hd)=0.125 score scale.
A_SCH = 0.125 * 1024.0 * 1.4426950408889634
B_SCH = 15.0 * 1024.0 - 60.0

_nc_cache = {}


# --------------------------------------------------------------------------
# BIR fix: this container's walrus supports only ONE sync wait (and update)
# per TPB instruction; Tile attaches several.  Split extras onto single-wait
# EventSemaphore instructions at the serialization boundary.
# --------------------------------------------------------------------------
_wsplit_counter = [0]


def _mk_evsem(engine, debug, wait=None, update=None):
    _wsplit_counter[0] += 1
    return {
        "debug": debug,
        "engine": engine,
        "ins": [],
        "outs": [],
        "name": f"wsplit-{_wsplit_counter[0]}",
        "opcode": "EventSemaphore",
        "sync_info": {
            "on_wait": [wait] if wait else [],
            "on_update": [update] if update else [],
        },
    }


def _split_bir_waits(bir):
    for f in bir.get("functions", []):
        for blk in f.get("blocks", []):
            out = []
            for inst in blk.get("instructions", []):
                si = inst.get("sync_info")
                waits = list(si.get("on_wait") or []) if si else []
                updates = list(si.get("on_update") or []) if si else []
                eng = inst.get("engine")
                dbg = inst.get("debug", 0)
                if len(waits) > 1:
                    for w in waits[:-1]:
                        out.append(_mk_evsem(eng, dbg, wait=w))
                    si["on_wait"] = [waits[-1]]
                out.append(inst)
                if len(updates) > 1:
                    si["on_update"] = [updates[0]]
                    for u in updates[1:]:
                        out.append(_mk_evsem(eng, dbg, update=u))
            blk["instructions"] = out
    return bir


def _install_bir_fix():
    import concourse.bass as bass

    if getattr(bass.Bass, "_wsplit_installed", False):
        return
    orig = bass.Bass.to_json_bytes

    def to_json_bytes(self, *a, **k):
        bir = json.loads(orig(self, *a, **k))
        return json.dumps(_split_bir_waits(bir)).encode()

    bass.Bass.to_json_bytes = to_json_bytes
    bass.Bass._wsplit_installed = True


# --------------------------------------------------------------------------
# Kernel builder
# --------------------------------------------------------------------------

def _build_nc():
    import concourse.bass as bass
    import concourse.mybir as mybir
    import concourse.tile as tile

    f16 = mybir.dt.float16
    i16 = mybir.dt.int16
    f32 = mybir.dt.float32
    f32r = mybir.dt.float32r
    Exp = mybir.ActivationFunctionType.Exp
    Ln = mybir.ActivationFunctionType.Ln

    nc = bass.Bass("TRN2")

    xq = nc.dram_tensor("xq", [H, S], f16, kind="ExternalInput")
    xk = nc.dram_tensor("xk", [H, S], f16, kind="ExternalInput")
    xv = nc.dram_tensor("xv", [H, S], f16, kind="ExternalInput")
    wqT = nc.dram_tensor("wqT", [H, DL], f16, kind="ExternalInput")
    wkT = nc.dram_tensor("wkT", [H, DL], f16, kind="ExternalInput")
    wvT = nc.dram_tensor("wvT", [H, DL], f16, kind="ExternalInput")
    woT = nc.dram_tensor("woT", [DL, H], f16, kind="ExternalInput")
    bias = nc.dram_tensor("bias", [P, 4], f32, kind="ExternalInput")  # bq0 bq1 bk0 bk1
    out = nc.dram_tensor("out", [H, S], f16, kind="ExternalOutput")

    with tile.TileContext(nc) as tc:
        with (
            tc.tile_pool(name="persist", bufs=1) as persist,
            tc.tile_pool(name="xpool", bufs=48) as xpool,
            tc.tile_pool(name="exppool", bufs=10) as exppool,
            tc.tile_pool(name="lrpool", bufs=3) as lrpool,
            tc.tile_pool(name="oevpool", bufs=2) as oevpool,
            tc.tile_pool(name="evpool", bufs=3) as evpool,
            tc.tile_pool(name="scps", bufs=2, space="PSUM") as scps,
            tc.tile_pool(name="avps", bufs=1, space="PSUM") as avps,
            tc.tile_pool(name="lps", bufs=1, space="PSUM") as lpsp,
            tc.tile_pool(name="auxps", bufs=2, space="PSUM") as auxps,
        ):
            # ---- persistent tiles ----
            wq_sb = persist.tile([P, NK, DL], f16, name="wq_sb")
            wk_sb = persist.tile([P, NK, DL], f16, name="wk_sb")
            wv_sb = persist.tile([P, NK, DL], f16, name="wv_sb")
            wo_sb = persist.tile([P, 2, H], f16, name="wo_sb")
            bias_sb = persist.tile([P, 4], f32, name="bias_sb")
            sel4 = persist.tile([P, P], f32r, name="sel4")
            ones16 = persist.tile([P, NB], f16, name="ones16")
            qpt = persist.tile([P, 2, S], f16, name="qpt")
            kpt = persist.tile([P, 2, S], f16, name="kpt")
            vp = persist.tile([P, NJ, DL], f16, name="vp")
            onorm = persist.tile([P, 2, S], f16, name="onorm")

            nc.sync.dma_start(wv_sb[:], wvT.rearrange("(k p) d -> p k d", p=P))
            nc.sync.dma_start(wk_sb[:], wkT.rearrange("(k p) d -> p k d", p=P))
            nc.sync.dma_start(wq_sb[:], wqT.rearrange("(k p) d -> p k d", p=P))
            nc.sync.dma_start(bias_sb[:], bias[:])
            # selector for the 1/L broadcast: rows 0/64 (query halves) ->
            # out partitions 0-63 (head A), rows 32/96 -> 64-127 (head B)
            sel_f = evpool.tile([P, P], f32, name="self_t")
            nc.gpsimd.memset(sel_f[:], 0.0)
            nc.gpsimd.memset(sel_f[0:1, 0:HD], 1.0)
            nc.gpsimd.memset(sel_f[32:33, HD:P], 1.0)
            nc.gpsimd.memset(sel_f[64:65, 0:HD], 1.0)
            nc.gpsimd.memset(sel_f[96:97, HD:P], 1.0)
            nc.vector.tensor_copy(sel4[:], sel_f[:])
            nc.gpsimd.memset(ones16[:], 1.0)

            # ---- PE warm-up: dense junk matmuls during the DMA lead-in so
            # the HAM clock-gate releases before real work arrives ----
            warm_ps = auxps.tile([P, NB], f32, name="aux_t")
            for w in range(36):
                nc.tensor.matmul(
                    warm_ps[:, 0:P], ones16[:, 0:P], ones16[:, 0:P],
                    start=(w == 0), stop=(w == 35),
                )

            # one-time fill of the L bank so rows never written by the L
            # matmuls hold a safe value (recip stays finite, selector is 0
            # there anyway)
            lp_init = lpsp.tile([P, NB], f32, name="lp_t")
            nc.vector.memset(lp_init[:], 1.0)

            # ---- x loads: [P, 2*NB] halves keyed (k, h) — 256 KB per DMA
            # descriptor (the Sync queue issues descriptors serially, so
            # fewer/larger transfers raise the input stream rate), all of x
            # resident.  (k, n) quarter lookups return (tile, col offset).
            xv_q = {}
            xk_q = {}
            xq_q = {}
            def load_h(dst, src, k, h, eng):
                t = xpool.tile([P, 2 * NB], f16, name="x_sb")
                eng.dma_start(
                    t[:], src[k * P:(k + 1) * P, h * 2 * NB:(h + 1) * 2 * NB])
                dst[(k, 2 * h)] = (t, 0)
                dst[(k, 2 * h + 1)] = (t, NB)

            # all x loads on the sync hardware-dynamic DMA queue; splitting
            # across the gpsimd (software-dynamic, slow) or scalar queues
            # measured strictly worse
            load_order = [
                (xk_q, xk, 0, nc.sync), (xq_q, xq, 0, nc.sync),
                (xv_q, xv, 0, nc.sync), (xk_q, xk, 1, nc.sync),
                (xv_q, xv, 1, nc.sync), (xq_q, xq, 1, nc.sync),
            ]
            for dst, src, h, eng in load_order:
                for k in range(NK):
                    load_h(dst, src, k, h, eng)
            nc.sync.dma_start(wo_sb[:], woT.rearrange("(k p) d -> p k d", p=P))

            # ---- V projection for one s-tile ----
            def v_block(s):
                n, c = divmod(s, 4)
                ps = auxps.tile([P, NB], f32, name="aux_t")
                for k in range(NK):
                    t, off = xv_q[(k, n)]
                    nc.tensor.matmul(
                        ps[:, :DL],
                        t[:, off + c * P:off + (c + 1) * P],
                        wv_sb[:, k, :],
                        start=(k == 0),
                        stop=(k == NK - 1),
                    )
                nc.vector.tensor_copy(vp[:, s, :], ps[:, :DL])

            # ---- K/Q projection, one n-block ----
            def proj_qk_n(wsb, xq_, dst, bcol, m, n):
                ps = auxps.tile([P, NB], f32, name="aux_t")
                for k in range(NK):
                    t, off = xq_[(k, n)]
                    nc.tensor.matmul(
                        ps[:],
                        wsb[:, k, m * P:(m + 1) * P],
                        t[:, off:off + NB],
                        start=(k == 0),
                        stop=(k == NK - 1),
                    )
                nc.vector.tensor_scalar_add(
                    dst[:, m, n * NB:(n + 1) * NB], ps[:],
                    bias_sb[:, bcol + m:bcol + m + 1],
                )

            # ---- one Wo output column block (i-block n, out-rows mo*128) ----
            def wo_mo(n, mo, deep=False):
                # deep=True (final serial tail): rotate PSUM across scps too
                # (free by then) for a 4-deep MM->cast->DMA pipeline
                if deep and mo % 2:
                    psa = scps.tile([P, 2 * NB], f32, name="sc_t")[:, 0:NB]
                else:
                    psa = auxps.tile([P, NB], f32, name="aux_t")[:]
                for k2 in range(2):
                    nc.tensor.matmul(
                        psa,
                        wo_sb[:, k2, mo * P:(mo + 1) * P],
                        onorm[:, k2, n * NB:(n + 1) * NB],
                        start=(k2 == 0),
                        stop=(k2 == 1),
                    )
                ot = evpool.tile([P, NB], f16, name="ot_t")
                if mo % 2:
                    # alternate PSUM evacuation between the scalar and
                    # vector engines so neither becomes the gate
                    nc.scalar.copy(ot[:], psa)
                else:
                    nc.vector.tensor_copy(ot[:], psa)
                nc.sync.dma_start(out[mo * P:(mo + 1) * P, n * NB:(n + 1) * NB], ot[:])

            # ---- normalization tail of one finished (p, ib): returned as a
            # list of closures to be interleaved into the next ib ----
            def make_tail(p, ib, av, lp):
                isl = slice(ib * NB, (ib + 1) * NB)
                HB = NB // 2
                lnb = lrpool.tile([P, NB], f32, name="lnb_t")
                rbs = lrpool.tile([P, NB], f32r, name="rbs_t")
                o_ev = oevpool.tile([P, NB], f16, name="oev_t")
                st = {}

                def c_evac():
                    nc.vector.tensor_copy(o_ev[:], av[:])

                def c_ln():
                    # rows 0/32/64/96 hold L for (A,B) x (q-lo,q-hi); the
                    # 1.0-filled filler rows come out as ln=0 -> 1/L=1 and
                    # are zeroed by the selector
                    nc.scalar.activation(lnb[0:97, 0:HB], lp[0:97, 0:HB], Ln)

                def c_exp():
                    # 1/L = exp(-ln L) on the scalar engine (97 partitions
                    # in parallel; the DVE reciprocal instruction is ~4.5x
                    # slower per column)
                    with nc.allow_low_precision(
                            reason="softmax denom via f32r"):
                        nc.scalar.activation(
                            rbs[0:97, 0:HB], lnb[0:97, 0:HB], Exp, scale=-1.0)

                def c_bcast():
                    # two row-tiled selector matmuls (contract at partitions
                    # 0-32 and 64-96) -> separate PSUM banks, one per
                    # 256-query half
                    rb0 = auxps.tile([P, NB], f32, name="aux_t")
                    rb1 = auxps.tile([P, NB], f32, name="aux_t")
                    nc.tensor.matmul(
                        rb0[:, 0:HB], sel4[0:33, :], rbs[0:33, 0:HB],
                        start=True, stop=True,
                    )
                    nc.tensor.matmul(
                        rb1[:, 0:HB], sel4[64:97, :], rbs[64:97, 0:HB],
                        start=True, stop=True,
                    )
                    st["rb0"], st["rb1"] = rb0, rb1

                def c_mul():
                    i0 = ib * NB
                    nc.vector.tensor_mul(
                        onorm[:, p, i0:i0 + HB], o_ev[:, 0:HB],
                        st["rb0"][:, 0:HB])
                    nc.vector.tensor_mul(
                        onorm[:, p, i0 + HB:i0 + NB], o_ev[:, HB:NB],
                        st["rb1"][:, 0:HB])

                return [c_evac, c_ln, c_exp, c_bcast, c_mul]

            # ---- attention over one (head-pair p, i-block ib) ----
            def attention_ib(p, ib, v_inline=False, mids=None, dve_js=(),
                             carry=(), final=False):
                isl = slice(ib * NB, (ib + 1) * NB)
                av = avps.tile([P, NB], f32, name="av_t")
                lp = lpsp.tile([P, NB], f32, name="lp_t")
                carry = list(carry)
                es = []
                pend = []
                a0 = (2 * p) * HD
                b0 = (2 * p + 1) * HD

                def emit_av(jb):
                    eA, eB = es[jb][0], es[jb][1]
                    for ci, (voff, ee) in enumerate(
                            ((a0, eA), (a0 + 32, eA), (b0, eB), (b0 + 32, eB))):
                        nc.tensor.matmul(
                            av[ci * 32:(ci + 1) * 32, :],
                            vp[:, jb, voff:voff + 32],
                            ee,
                            start=(jb == 0), stop=(jb == NJ - 1),
                            tile_position=(0, ci * 32),
                        )

                def emit_L(jb):
                    # column-tiled m=1 rider pass over 256-query halves:
                    # (A,q-lo)@0 (B,q-lo)@32 (A,q-hi)@64 (B,q-hi)@96, each
                    # accumulating over all 16 j-tiles
                    HB = NB // 2
                    for ci, ee in enumerate(es[jb][2:6]):
                        nc.tensor.matmul(
                            lp[ci * 32:ci * 32 + 1, 0:HB],
                            ones16[:, 0:1],
                            ee,
                            start=(jb == 0), stop=(jb == NJ - 1),
                            tile_position=(0, ci * 32),
                        )

                for jj in range(0, NJ, 2):
                    for jb in (jj, jj + 1):
                        jsl = slice(jb * P, (jb + 1) * P)
                        sc = scps.tile([P, 2 * NB], f32, name="sc_t")
                        nc.tensor.matmul(
                            sc[:, 0:NB], kpt[0:HD, p, jsl], qpt[0:HD, p, isl],
                            start=True, stop=True,
                        )
                        nc.tensor.matmul(
                            sc[:, NB:2 * NB], kpt[HD:P, p, jsl], qpt[HD:P, p, isl],
                            start=True, stop=True,
                        )
                        HB = NB // 2
                        cuts = ((0, NB), (NB, 2 * NB), (0, HB), (NB, NB + HB),
                                (HB, NB), (NB + HB, 2 * NB))
                        if jb in dve_js:
                            e = exppool.tile([P, 2 * NB], i16, name="e_t")
                            nc.vector.tensor_scalar(
                                e[:], sc[:], float(A_SCH), float(B_SCH),
                                mybir.AluOpType.mult, mybir.AluOpType.add,
                            )
                            es.append(tuple(
                                e[:, lo:hi].bitcast(f16) for lo, hi in cuts))
                        else:
                            e = exppool.tile([P, 2 * NB], f16, name="e_t")
                            nc.scalar.activation(e[:], sc[:], Exp, scale=0.125)
                            es.append(tuple(
                                e[:, lo:hi] for lo, hi in cuts))
                        if v_inline and 4 <= jb:
                            v_block(jb - 2)
                        if mids is not None and jb in mids:
                            pend.extend(mids[jb])
                    # carried-in PE work (prev ib's trailing AV/L passes or
                    # lead-in v-blocks) fills the pipeline-refill bubble
                    for _ in range(4):
                        if carry:
                            carry.pop(0)()
                    # AV+L flush: one contiguous column-tiled run
                    if jj in (6, 10, 14):
                        base = jj - 6
                        for j2 in range(base, base + 4):
                            emit_av(j2)
                        for j2 in range(base, base + 4):
                            emit_L(j2)
                    if final and jj == 14:
                        emit_av(12)
                        emit_av(13)
                        emit_L(12)
                        emit_L(13)
                    # mid callbacks run at the block boundary, after the
                    # column-tiled flush, so proj/wo full-array matmuls do
                    # not fragment the row-tiled QK runs (each fragment
                    # costs a PE tiling-mode drain)
                    for cb in pend:
                        cb()
                    pend = []

                if v_inline:
                    v_block(NJ - 2)
                    v_block(NJ - 1)
                if final:
                    trailing = [lambda: emit_av(14), lambda: emit_av(15),
                                lambda: emit_L(14), lambda: emit_L(15)]
                else:
                    trailing = [
                        (lambda a=jb: emit_av(a))
                        for jb in range(NJ - 4, NJ)
                    ] + [(lambda a=jb: emit_L(a)) for jb in range(NJ - 4, NJ)]
                return make_tail(p, ib, av, lp), trailing

            # ---- schedule ----
            def P_(wsb, xd, dst, bcol, m, n):
                return lambda: proj_qk_n(wsb, xd, dst, bcol, m, n)

            K0 = lambda n: P_(wk_sb, xk_q, kpt, 2, 0, n)
            Q0 = lambda n: P_(wq_sb, xq_q, qpt, 0, 0, n)
            K1 = lambda n: P_(wk_sb, xk_q, kpt, 2, 1, n)
            Q1 = lambda n: P_(wq_sb, xq_q, qpt, 0, 1, n)

            def sched(tail, extra=None, wo_n=None):
                """mids dict: tail closures (evac@3, ln@4, exp@5, bcast@7,
                mul@9), wo pieces 2-per-slot at 12..15, plus extras.  The
                prev ib's av/lp only complete once its carried trailing
                passes ran (slots 0..3)."""
                m = {}
                if tail is not None:
                    slots = (3, 4, 5, 7, 9)
                    for i, cb in enumerate(tail):
                        m.setdefault(slots[i], []).append(cb)
                if wo_n is not None:
                    for mo in range(8):
                        m.setdefault(8 + mo, []).append(
                            (lambda n_, mo_: lambda: wo_mo(n_, mo_))(wo_n, mo))
                if extra:
                    for s, cbs in extra.items():
                        m.setdefault(s, []).extend(cbs)
                return m

            # DVE-exp tile positions chosen so the scalar engine never gets
            # more than two consecutive j-tiles (3 back-to-back ACT exps =
            # 3.3us serial vs ~1.7us of PE work -> per-ib PE stall)
            D5 = (2, 5, 8, 11, 13)    # final ib
            D5L = (2, 5, 8, 11, 14)   # non-wo ibs
            DW = (2, 5, 8, 11, 13, 15)  # wo ibs: one more DVE tile

            K0(0)()
            Q0(0)()
            t, cav = attention_ib(0, 0, v_inline=True, mids=sched(
                None, extra={2: [K0(1)], 5: [K0(2)], 8: [K0(3)], 11: [Q0(1)]}),
                carry=[lambda s=s: v_block(s) for s in range(2)])
            t, cav = attention_ib(0, 1, mids=sched(
                t, extra={5: [Q0(2)], 9: [K1(0)], 13: [K1(1)]}),
                dve_js=D5L, carry=cav)
            t, cav = attention_ib(0, 2, mids=sched(
                t, extra={7: [Q0(3)], 11: [Q1(0)], 14: [Q1(1)]}),
                dve_js=D5L, carry=cav)
            t, cav = attention_ib(1, 0, mids=sched(
                t, extra={2: [K1(2)], 5: [K1(3)], 9: [Q1(2)]}),
                dve_js=D5L, carry=cav)
            t, cav = attention_ib(1, 1, mids=sched(
                t, extra={9: [Q1(3)]}), dve_js=DW, carry=cav)
            t, cav = attention_ib(1, 2, mids=sched(t, wo_n=0), dve_js=DW,
                                  carry=cav)
            t, cav = attention_ib(1, 3, mids=sched(t, wo_n=1), dve_js=DW,
                                  carry=cav)
            t, cav = attention_ib(0, 3, mids=sched(t, wo_n=2), dve_js=D5,
                                  carry=cav, final=True)
            # final serial tail
            for cb in cav:
                cb()
            for cb in t:
                cb()
            for mo in range(8):
                wo_mo(3, mo, deep=True)

    return nc


def _get_nc():
    if "nc" not in _nc_cache:
        _install_bir_fix()
        _nc_cache["nc"] = _build_nc()
    return _nc_cache["nc"]


# --------------------------------------------------------------------------
# Host wrapper
# --------------------------------------------------------------------------
def run(inputs, trace=False):
    from concourse.bass_utils import run_bass_kernel_spmd

    q = np.asarray(inputs["q"], np.float32)
    k = np.asarray(inputs["k"], np.float32)
    v = np.asarray(inputs["v"], np.float32)
    Wq = np.asarray(inputs["Wq"], np.float32)
    bq = np.asarray(inputs["bq"], np.float32)
    Wk = np.asarray(inputs["Wk"], np.float32)
    bk = np.asarray(inputs["bk"], np.float32)
    Wv = np.asarray(inputs["Wv"], np.float32)
    bv = np.asarray(inputs["bv"], np.float32)
    Wo = np.asarray(inputs["Wo"], np.float32)
    bo = np.asarray(inputs["bo"], np.float32)

    nc = _get_nc()

    xT = {}
    for b in range(2):
        xT[b] = (
            np.ascontiguousarray(q[b].T).astype(np.float16),
            np.ascontiguousarray(k[b].T).astype(np.float16),
            np.ascontiguousarray(v[b].T).astype(np.float16),
        )

    in_maps = []
    for c in range(8):
        b, g = divmod(c, 4)
        sl = slice(g * DL, (g + 1) * DL)
        bias = np.stack(
            [bq[sl][:P], bq[sl][P:], bk[sl][:P], bk[sl][P:]], axis=1
        ).astype(np.float32)
        in_maps.append({
            "xq": xT[b][0],
            "xk": xT[b][1],
            "xv": xT[b][2],
            "wqT": np.ascontiguousarray(Wq[sl, :].T).astype(np.float16),
            "wkT": np.ascontiguousarray(Wk[sl, :].T).astype(np.float16),
            "wvT": np.ascontiguousarray(Wv[sl, :].T).astype(np.float16),
            "woT": np.ascontiguousarray(Wo[:, sl].T).astype(np.float16),
            "bias": bias,
        })

    res = run_bass_kernel_spmd(
        nc, in_maps, core_ids=list(range(8)), trace=trace,
    )
    outs = [r["out"] for r in res.results]

    const = (Wo @ bv + bo).astype(np.float32)  # [1024]
    full = np.empty((2, S, H), np.float32)
    for b in range(2):
        acc = outs[4 * b].astype(np.float32).copy()
        for g in range(1, 4):
            acc += outs[4 * b + g]
        full[b] = acc.T + const
    return full, res


def kernel(**inputs):
    full, _ = run(inputs, trace=False)
    return full
